# revision 9
# baseline (speedup 1.0000x reference)
"""GRU (r_t=1) Trainium2 kernel v3: batch-sharded, fused single pass.

On top of v2:
- iz'/in' are injected into PSUM by identity matmuls at the start of each
  step (start=True), so the W_h matmuls accumulate onto them and the DVE
  adds (and their semaphore hops) disappear; tanh reads PSUM directly.
  The injection matmuls have no h-dependency, so they also fill the PE
  idle gap while the previous step's tail completes (keeps HAM warm).
- The z-gate PSUM is split across two banks (hidden chunks 0-1 / 2-3) so
  the tanh+blend for chunks 0-1 runs while the PE still computes chunks
  2-3, and the next step's k=0,1 matmuls start as soon as h' chunks 0-1
  are written.
"""

import sys

if "/opt/trn_rl_repo" not in sys.path:
    sys.path.insert(0, "/opt/trn_rl_repo")

from contextlib import ExitStack

import ml_dtypes
import numpy as np

import concourse.bacc as bacc
import concourse.bass as bass
import concourse.mybir as mybir
import concourse.tile as tile
from concourse import bass_utils

NCORES = 8
DIN = 512
DH = 512
CH = DH // 128  # 4 hidden chunks of 128 partitions
AF = mybir.ActivationFunctionType
ALU = mybir.AluOpType

BF16 = mybir.dt.bfloat16
F32 = mybir.dt.float32
NP_BF16 = np.dtype(ml_dtypes.bfloat16)


def build_nc(T: int, BC: int):
    """Build the per-core Bass program. BC = batch per core."""
    R = T * BC
    BLK = 64
    while T % BLK:
        BLK //= 2
    NBLK = T // BLK
    BCOL = BLK * BC  # columns per scan block
    NCC = BCOL // 512  # 512-column projection chunks per block
    assert BCOL % 512 == 0
    NUNIT = 2 * CH * NCC  # projection units per block (gate x m-chunk x chunk)
    HALF = CH // 2

    nc = bacc.Bacc("TRN2", target_bir_lowering=False, debug=False)

    seqT = nc.dram_tensor("seqT", [DIN, R], BF16, kind="ExternalInput").ap()
    wizT = nc.dram_tensor("wizT", [DIN, DH], BF16, kind="ExternalInput").ap()
    winT = nc.dram_tensor("winT", [DIN, DH], BF16, kind="ExternalInput").ap()
    whzT = nc.dram_tensor("whzT", [DH, DH], BF16, kind="ExternalInput").ap()
    whnT = nc.dram_tensor("whnT", [DH, DH], BF16, kind="ExternalInput").ap()
    ident = nc.dram_tensor("ident", [128, 128], BF16, kind="ExternalInput").ap()
    biasz = nc.dram_tensor("biasz", [128, CH], F32, kind="ExternalInput").ap()
    biasn = nc.dram_tensor("biasn", [128, CH], F32, kind="ExternalInput").ap()
    HT = nc.dram_tensor("HT", [CH, 128, R], BF16, kind="ExternalOutput").ap()

    with tile.TileContext(nc) as tc, ExitStack() as ctx:
        const = ctx.enter_context(tc.tile_pool(name="const", bufs=1))

        wiz_sb = const.tile([128, CH, DH], BF16)
        win_sb = const.tile([128, CH, DH], BF16)
        whz_sb = const.tile([128, CH, DH], BF16)
        whn_sb = const.tile([128, CH, DH], BF16)
        id_sb = const.tile([128, 128], BF16)
        for sb, dr in ((wiz_sb, wizT), (win_sb, winT), (whz_sb, whzT), (whn_sb, whnT)):
            nc.gpsimd.dma_start(sb[:], dr.rearrange("(c p) h -> p c h", p=128))
        nc.gpsimd.dma_start(id_sb[:], ident[:])
        bz_sb = const.tile([128, CH], F32)
        bn_sb = const.tile([128, CH], F32)
        nc.gpsimd.dma_start(bz_sb[:], biasz[:])
        nc.gpsimd.dma_start(bn_sb[:], biasn[:])

        junk_ps = ctx.enter_context(tc.tile_pool(name="junkps", bufs=1, space="PSUM"))
        junk = junk_ps.tile([128, 8], F32)
        scratch = const.tile([128, 8], F32)

        def pe_touch(ap_k1):
            nc.tensor.matmul(junk[0:1, 0:1], ap_k1, ap_k1, start=True, stop=True)

        for sb in (wiz_sb, win_sb, whz_sb, whn_sb, id_sb):
            pe_touch(sb[:, 0, 0:1] if sb is not id_sb else sb[:, 0:1])
        nc.scalar.copy(scratch[0:1, 0:1], bz_sb[0:1, 0:1])
        nc.scalar.copy(scratch[0:1, 1:2], bn_sb[0:1, 1:2])

        with (
            tc.tile_pool(name="seqp", bufs=2 * NCC) as seq_pool,
            tc.tile_pool(name="izinp", bufs=2) as izin_pool,
            tc.tile_pool(name="ht2", bufs=2) as ht_pool,
            tc.tile_pool(name="ew2", bufs=3) as ew_pool,
            tc.tile_pool(name="psum2", bufs=2, space="PSUM") as psum2,
            tc.tile_pool(name="psump", bufs=1, space="PSUM") as psum_p,
        ):

            def load_seq_block(blk):
                tiles = []
                for cc in range(NCC):
                    csl = slice(blk * BCOL + cc * 512, blk * BCOL + (cc + 1) * 512)
                    sq = seq_pool.tile([128, CH, 512], BF16, tag="sq")
                    nc.gpsimd.dma_start(
                        sq[:], seqT[:, csl].rearrange("(c p) r -> p c r", p=128)
                    )
                    pe_touch(sq[:, 0, 0:1])
                    tiles.append(sq)
                return tiles

            TPC = 512 // BC  # timesteps covered by one projection unit

            def emit_proj_unit(izin_dst, sq_tiles, unit):
                cc, rem = divmod(unit, 2 * CH)
                g, m = divmod(rem, CH)
                w_sb, b_sb = (wiz_sb, bz_sb) if g == 0 else (win_sb, bn_sb)
                sq = sq_tiles[cc]
                ps = psum_p.tile([128, TPC, BC], F32, tag="psp")
                for k in range(CH):
                    nc.tensor.matmul(
                        ps[:],
                        w_sb[:, k, m * 128 : (m + 1) * 128],
                        sq[:, k, :],
                        start=(k == 0),
                        stop=(k == CH - 1),
                    )
                nc.scalar.activation(
                    izin_dst[:, g, cc * TPC : (cc + 1) * TPC, m, :],
                    ps[:],
                    AF.Identity,
                    bias=b_sb[:, m : m + 1],
                    scale=1.0,
                )

            # Prologue: block 0 projections, then start block 1's seq loads.
            # izin is time-major [p, gate, step, chunk, batch] so each step's
            # injection matmul reads a fully contiguous [128, CH, BC] slab.
            sq_cur = load_seq_block(0)
            izin_cur = izin_pool.tile([128, 2, BLK, CH, BC], BF16, tag="izin")
            for unit in range(NUNIT):
                emit_proj_unit(izin_cur, sq_cur, unit)

            h0 = const.tile([128, CH, BC], BF16)
            nc.vector.memset(h0[:], 0.0)
            hprev = h0[:]

            for blk in range(NBLK):
                bsl = slice(blk * BCOL, (blk + 1) * BCOL)
                htb = ht_pool.tile([128, CH, BCOL], BF16, tag="htb")
                if blk + 1 < NBLK:
                    sq_next = load_seq_block(blk + 1)
                    izin_next = izin_pool.tile([128, 2, BLK, CH, BC], BF16, tag="izin")
                else:
                    sq_next = izin_next = None

                first_unit_step = max(0, BLK - 3 * NUNIT)

                for tl in range(BLK):
                    tsl = slice(tl * BC, (tl + 1) * BC)
                    psn = psum2.tile([128, CH, BC], F32, tag="psn")
                    psz01 = psum2.tile([128, HALF, BC], F32, tag="psz01")
                    psz23 = psum2.tile([128, HALF, BC], F32, tag="psz23")

                    def zps(m):
                        return psz01[:, m, :] if m < HALF else psz23[:, m - HALF, :]

                    # Inject iz'/in' into PSUM: ONE matmul per bank covering
                    # the bank's whole written region (start=True clears the
                    # has_written bits bank-wide, so there must be exactly one
                    # start per bank, issued first). No h dependency: fills
                    # the PE gap while the previous step's tail completes.
                    nc.tensor.matmul(
                        psn[:], id_sb[:], izin_cur[:, 1, tl, :, :],
                        start=True, stop=False, skip_group_check=True,
                    )
                    nc.tensor.matmul(
                        psz01[:], id_sb[:], izin_cur[:, 0, tl, 0:HALF, :],
                        start=True, stop=False, skip_group_check=True,
                    )
                    nc.tensor.matmul(
                        psz23[:], id_sb[:], izin_cur[:, 0, tl, HALF:CH, :],
                        start=True, stop=False, skip_group_check=True,
                    )
                    # n gate first: its tail (tanh, h-n) hides under the
                    # z-gate matmuls. k-outer order so the k=0,1 matmuls
                    # (gated on the early h' half) all issue before k=2,3.
                    for k in range(CH):
                        for m in range(CH):
                            nc.tensor.matmul(
                                psn[:, m, :],
                                whn_sb[:, k, m * 128 : (m + 1) * 128],
                                hprev[:, k, :],
                                start=False,
                                stop=(k == CH - 1),
                                skip_group_check=True,
                            )
                    # Touch the last-written psn element on DVE: forces a PE
                    # semaphore increment right after psn completes, so tanh_n
                    # fires mid-burst instead of after the whole z burst.
                    nc.vector.tensor_copy(
                        scratch[0:1, 4:5], psn[0:1, CH - 1, BC - 1 : BC]
                    )
                    for k in range(CH):
                        for m in range(HALF):
                            nc.tensor.matmul(
                                zps(m),
                                whz_sb[:, k, m * 128 : (m + 1) * 128],
                                hprev[:, k, :],
                                start=False,
                                stop=(k == CH - 1),
                                skip_group_check=True,
                            )
                    nc.vector.tensor_copy(
                        scratch[0:1, 5:6], psz01[0:1, HALF - 1, BC - 1 : BC]
                    )
                    for k in range(CH):
                        for m in range(HALF, CH):
                            nc.tensor.matmul(
                                zps(m),
                                whz_sb[:, k, m * 128 : (m + 1) * 128],
                                hprev[:, k, :],
                                start=False,
                                stop=(k == CH - 1),
                                skip_group_check=True,
                            )
                    n_t = ew_pool.tile([128, CH, BC], F32, tag="n_t")
                    nc.scalar.activation(n_t[:], psn[:], AF.Tanh)
                    d = ew_pool.tile([128, CH, BC], F32, tag="d")
                    nc.vector.tensor_sub(d[:], hprev[:], n_t[:])
                    # z path, split in half by hidden chunk so the first half
                    # of h' lands early and unblocks the next step's k=0,1.
                    u = ew_pool.tile([128, CH, BC], F32, tag="u")
                    t1 = ew_pool.tile([128, CH, BC], F32, tag="t1")
                    nc.scalar.activation(u[:, 0:HALF, :], psz01[:], AF.Tanh)
                    nc.vector.scalar_tensor_tensor(
                        t1[:, 0:HALF, :], u[:, 0:HALF, :], 1.0, d[:, 0:HALF, :],
                        ALU.add, ALU.mult,
                    )
                    nc.vector.scalar_tensor_tensor(
                        htb[:, 0:HALF, tsl], t1[:, 0:HALF, :], 0.5, n_t[:, 0:HALF, :],
                        ALU.mult, ALU.add,
                    )
                    nc.scalar.activation(u[:, HALF:CH, :], psz23[:], AF.Tanh)
                    nc.vector.scalar_tensor_tensor(
                        t1[:, HALF:CH, :], u[:, HALF:CH, :], 1.0, d[:, HALF:CH, :],
                        ALU.add, ALU.mult,
                    )
                    nc.vector.scalar_tensor_tensor(
                        htb[:, HALF:CH, tsl], t1[:, HALF:CH, :], 0.5,
                        n_t[:, HALF:CH, :], ALU.mult, ALU.add,
                    )
                    hprev = htb[:, :, tsl]

                    if (
                        izin_next is not None
                        and tl >= first_unit_step
                        and (tl - first_unit_step) % 3 == 0
                    ):
                        unit = (tl - first_unit_step) // 3
                        if unit < NUNIT:
                            emit_proj_unit(izin_next, sq_next, unit)

                nc.gpsimd.dma_start(HT[:, :, bsl].rearrange("c p r -> p c r"), htb[:])
                izin_cur = izin_next
                sq_cur = sq_next

    nc.compile()
    return nc


_CACHE: dict = {}


def _get_nc(T, BC):
    key = (T, BC)
    if key not in _CACHE:
        _CACHE[key] = build_nc(T, BC)
    return _CACHE[key]


def make_in_maps(seq, W_iz, b_iz, W_in, b_in, W_hz, b_hz, W_hn, b_hn):
    T, B, _ = seq.shape
    BC = B // NCORES
    f32 = np.float32

    # z gate runs at half scale so z = (tanh(a/2)+1)/2 needs only Tanh.
    wizT = np.ascontiguousarray((W_iz.T * 0.5).astype(NP_BF16))
    winT = np.ascontiguousarray(W_in.T.astype(NP_BF16))
    whzT = np.ascontiguousarray((W_hz.T * 0.5).astype(NP_BF16))
    whnT = np.ascontiguousarray(W_hn.T.astype(NP_BF16))
    biasz = np.ascontiguousarray(((b_iz + b_hz) * 0.5).astype(f32).reshape(CH, 128).T)
    biasn = np.ascontiguousarray((b_in + b_hn).astype(f32).reshape(CH, 128).T)
    identity = np.ascontiguousarray(np.eye(128, dtype=np.float32).astype(NP_BF16))

    in_maps = []
    for c in range(NCORES):
        shard = seq[:, c * BC : (c + 1) * BC, :].reshape(T * BC, DIN)
        seqT = np.ascontiguousarray(shard.T.astype(NP_BF16))
        in_maps.append(
            {
                "seqT": seqT,
                "wizT": wizT,
                "winT": winT,
                "whzT": whzT,
                "whnT": whnT,
                "ident": identity,
                "biasz": biasz,
                "biasn": biasn,
            }
        )
    return in_maps


def run(inputs: dict, trace: bool = False):
    seq = inputs["seq"]
    T, B, _ = seq.shape
    BC = B // NCORES
    nc = _get_nc(T, BC)
    in_maps = make_in_maps(**inputs)
    return nc, bass_utils.run_bass_kernel_spmd(
        nc, in_maps, list(range(NCORES)), trace=trace
    )


def kernel(seq, W_iz, b_iz, W_in, b_in, W_hz, b_hz, W_hn, b_hn):
    T, B, _ = seq.shape
    BC = B // NCORES
    _, res = run(
        dict(
            seq=seq,
            W_iz=W_iz,
            b_iz=b_iz,
            W_in=W_in,
            b_in=b_in,
            W_hz=W_hz,
            b_hz=b_hz,
            W_hn=W_hn,
            b_hn=b_hn,
        )
    )
    out = np.empty((T, B, DH), np.float32)
    for c in range(NCORES):
        HT = res.results[c]["HT"]  # [CH, 128, T*BC] bf16
        Hc = (
            HT.astype(np.float32)
            .reshape(CH, 128, T, BC)
            .transpose(2, 3, 0, 1)
            .reshape(T, BC, DH)
        )
        out[:, c * BC : (c + 1) * BC, :] = Hc
    return out[None]


# revision 11
# speedup vs baseline: 1.1792x; 1.1792x over previous
"""GRU (r_t=1) Trainium2 kernel v3: batch-sharded, fused single pass.

On top of v2:
- iz'/in' are injected into PSUM by identity matmuls at the start of each
  step (start=True), so the W_h matmuls accumulate onto them and the DVE
  adds (and their semaphore hops) disappear; tanh reads PSUM directly.
  The injection matmuls have no h-dependency, so they also fill the PE
  idle gap while the previous step's tail completes (keeps HAM warm).
- The z-gate PSUM is split across two banks (hidden chunks 0-1 / 2-3) so
  the tanh+blend for chunks 0-1 runs while the PE still computes chunks
  2-3, and the next step's k=0,1 matmuls start as soon as h' chunks 0-1
  are written.
"""

import sys

if "/opt/trn_rl_repo" not in sys.path:
    sys.path.insert(0, "/opt/trn_rl_repo")

from contextlib import ExitStack

import ml_dtypes
import numpy as np

import concourse.bacc as bacc
import concourse.bass as bass
import concourse.mybir as mybir
import concourse.tile as tile
from concourse import bass_utils

NCORES = 8
DIN = 512
DH = 512
CH = DH // 128  # 4 hidden chunks of 128 partitions
AF = mybir.ActivationFunctionType
ALU = mybir.AluOpType

BF16 = mybir.dt.bfloat16
F32 = mybir.dt.float32
NP_BF16 = np.dtype(ml_dtypes.bfloat16)


def build_nc(T: int, BC: int):
    """Build the per-core Bass program. BC = batch per core."""
    R = T * BC
    BLK = 64
    while T % BLK:
        BLK //= 2
    NBLK = T // BLK
    BCOL = BLK * BC  # columns per scan block
    NCC = BCOL // 512  # 512-column projection chunks per block
    assert BCOL % 512 == 0
    NUNIT = 2 * CH * NCC  # projection units per block (gate x m-chunk x chunk)
    HALF = CH // 2

    nc = bacc.Bacc("TRN2", target_bir_lowering=False, debug=False)

    seqT = nc.dram_tensor("seqT", [DIN, R], BF16, kind="ExternalInput").ap()
    wizT = nc.dram_tensor("wizT", [DIN, DH], BF16, kind="ExternalInput").ap()
    winT = nc.dram_tensor("winT", [DIN, DH], BF16, kind="ExternalInput").ap()
    whzT = nc.dram_tensor("whzT", [DH, DH], BF16, kind="ExternalInput").ap()
    whnT = nc.dram_tensor("whnT", [DH, DH], BF16, kind="ExternalInput").ap()
    ident = nc.dram_tensor("ident", [128, 128], BF16, kind="ExternalInput").ap()
    biasz = nc.dram_tensor("biasz", [128, CH], F32, kind="ExternalInput").ap()
    biasn = nc.dram_tensor("biasn", [128, CH], F32, kind="ExternalInput").ap()
    HT = nc.dram_tensor("HT", [CH, 128, R], BF16, kind="ExternalOutput").ap()

    with tile.TileContext(nc) as tc, ExitStack() as ctx:
        const = ctx.enter_context(tc.tile_pool(name="const", bufs=1))

        wiz_sb = const.tile([128, CH, DH], BF16)
        win_sb = const.tile([128, CH, DH], BF16)
        whz_sb = const.tile([128, CH, DH], BF16)
        whn_sb = const.tile([128, CH, DH], BF16)
        id_sb = const.tile([128, 128], BF16)
        for sb, dr in ((wiz_sb, wizT), (win_sb, winT), (whz_sb, whzT), (whn_sb, whnT)):
            nc.gpsimd.dma_start(sb[:], dr.rearrange("(c p) h -> p c h", p=128))
        nc.gpsimd.dma_start(id_sb[:], ident[:])
        bz_sb = const.tile([128, CH], F32)
        bn_sb = const.tile([128, CH], F32)
        nc.gpsimd.dma_start(bz_sb[:], biasz[:])
        nc.gpsimd.dma_start(bn_sb[:], biasn[:])

        junk_ps = ctx.enter_context(tc.tile_pool(name="junkps", bufs=1, space="PSUM"))
        junk = junk_ps.tile([128, 8], F32)
        scratch = const.tile([128, 8], F32)

        def pe_touch(ap_k1):
            nc.tensor.matmul(junk[0:1, 0:1], ap_k1, ap_k1, start=True, stop=True)

        for sb in (wiz_sb, win_sb, whz_sb, whn_sb, id_sb):
            pe_touch(sb[:, 0, 0:1] if sb is not id_sb else sb[:, 0:1])
        nc.scalar.copy(scratch[0:1, 0:1], bz_sb[0:1, 0:1])
        nc.scalar.copy(scratch[0:1, 1:2], bn_sb[0:1, 1:2])

        with (
            tc.tile_pool(name="seqp", bufs=2 * NCC) as seq_pool,
            tc.tile_pool(name="izinp", bufs=2) as izin_pool,
            tc.tile_pool(name="ht2", bufs=2) as ht_pool,
            tc.tile_pool(name="ew2", bufs=3) as ew_pool,
            tc.tile_pool(name="psum2", bufs=2, space="PSUM") as psum2,
            tc.tile_pool(name="psump", bufs=1, space="PSUM") as psum_p,
        ):

            def load_seq_block(blk):
                tiles = []
                for cc in range(NCC):
                    csl = slice(blk * BCOL + cc * 512, blk * BCOL + (cc + 1) * 512)
                    sq = seq_pool.tile([128, CH, 512], BF16, tag="sq")
                    nc.gpsimd.dma_start(
                        sq[:], seqT[:, csl].rearrange("(c p) r -> p c r", p=128)
                    )
                    pe_touch(sq[:, 0, 0:1])
                    tiles.append(sq)
                return tiles

            TPC = 512 // BC  # timesteps covered by one projection unit

            def emit_proj_unit(izin_dst, sq_tiles, unit):
                cc, rem = divmod(unit, 2 * CH)
                g, m = divmod(rem, CH)
                w_sb, b_sb = (wiz_sb, bz_sb) if g == 0 else (win_sb, bn_sb)
                sq = sq_tiles[cc]
                ps = psum_p.tile([128, TPC, BC], F32, tag="psp")
                for k in range(CH):
                    nc.tensor.matmul(
                        ps[:],
                        w_sb[:, k, m * 128 : (m + 1) * 128],
                        sq[:, k, :],
                        start=(k == 0),
                        stop=(k == CH - 1),
                    )
                nc.scalar.activation(
                    izin_dst[:, g, cc * TPC : (cc + 1) * TPC, m, :],
                    ps[:],
                    AF.Identity,
                    bias=b_sb[:, m : m + 1],
                    scale=1.0,
                )

            # Prologue: block 0 projections, then start block 1's seq loads.
            # izin is time-major [p, gate, step, chunk, batch] so each step's
            # injection matmul reads a fully contiguous [128, CH, BC] slab.
            sq_cur = load_seq_block(0)
            izin_cur = izin_pool.tile([128, 2, BLK, CH, BC], BF16, tag="izin")
            for unit in range(NUNIT):
                emit_proj_unit(izin_cur, sq_cur, unit)

            h0 = const.tile([128, CH, BC], BF16)
            nc.vector.memset(h0[:], 0.0)
            hprev = h0[:]

            for blk in range(NBLK):
                bsl = slice(blk * BCOL, (blk + 1) * BCOL)
                htb = ht_pool.tile([128, CH, BCOL], BF16, tag="htb")
                if blk + 1 < NBLK:
                    sq_next = load_seq_block(blk + 1)
                    izin_next = izin_pool.tile([128, 2, BLK, CH, BC], BF16, tag="izin")
                else:
                    sq_next = izin_next = None

                first_unit_step = max(0, BLK - 3 * NUNIT)

                for tl in range(BLK):
                    tsl = slice(tl * BC, (tl + 1) * BC)
                    psn = psum2.tile([128, CH, BC], F32, tag="psn")
                    psz01 = psum2.tile([128, HALF, BC], F32, tag="psz01")
                    psz23 = psum2.tile([128, HALF, BC], F32, tag="psz23")

                    def zps(m):
                        return psz01[:, m, :] if m < HALF else psz23[:, m - HALF, :]

                    # Inject iz'/in' into PSUM: ONE matmul per bank covering
                    # the bank's whole written region (start=True clears the
                    # has_written bits bank-wide, so there must be exactly one
                    # start per bank, issued first). No h dependency: fills
                    # the PE gap while the previous step's tail completes.
                    nc.tensor.matmul(
                        psn[:], id_sb[:], izin_cur[:, 1, tl, :, :],
                        start=True, stop=False, skip_group_check=True,
                    )
                    nc.tensor.matmul(
                        psz01[:], id_sb[:], izin_cur[:, 0, tl, 0:HALF, :],
                        start=True, stop=False, skip_group_check=True,
                    )
                    nc.tensor.matmul(
                        psz23[:], id_sb[:], izin_cur[:, 0, tl, HALF:CH, :],
                        start=True, stop=False, skip_group_check=True,
                    )
                    # n gate first: its tail (tanh, h-n) hides under the
                    # z-gate matmuls. k-outer order so the k=0,1 matmuls
                    # (gated on the early h' half) all issue before k=2,3.
                    for k in range(CH):
                        for m in range(CH):
                            nc.tensor.matmul(
                                psn[:, m, :],
                                whn_sb[:, k, m * 128 : (m + 1) * 128],
                                hprev[:, k, :],
                                start=False,
                                stop=(k == CH - 1),
                                skip_group_check=True,
                            )
                    for k in range(CH):
                        for m in range(HALF):
                            nc.tensor.matmul(
                                zps(m),
                                whz_sb[:, k, m * 128 : (m + 1) * 128],
                                hprev[:, k, :],
                                start=False,
                                stop=(k == CH - 1),
                                skip_group_check=True,
                            )
                    for k in range(CH):
                        for m in range(HALF, CH):
                            nc.tensor.matmul(
                                zps(m),
                                whz_sb[:, k, m * 128 : (m + 1) * 128],
                                hprev[:, k, :],
                                start=False,
                                stop=(k == CH - 1),
                                skip_group_check=True,
                            )
                    n_t = ew_pool.tile([128, CH, BC], F32, tag="n_t")
                    nc.scalar.activation(n_t[:], psn[:], AF.Tanh)
                    d = ew_pool.tile([128, CH, BC], F32, tag="d")
                    nc.vector.tensor_sub(d[:], hprev[:], n_t[:])
                    # z path, split in half by hidden chunk so the first half
                    # of h' lands early and unblocks the next step's k=0,1.
                    u = ew_pool.tile([128, CH, BC], F32, tag="u")
                    t1 = ew_pool.tile([128, CH, BC], F32, tag="t1")
                    nc.scalar.activation(u[:, 0:HALF, :], psz01[:], AF.Tanh)
                    nc.vector.scalar_tensor_tensor(
                        t1[:, 0:HALF, :], u[:, 0:HALF, :], 1.0, d[:, 0:HALF, :],
                        ALU.add, ALU.mult,
                    )
                    nc.vector.scalar_tensor_tensor(
                        htb[:, 0:HALF, tsl], t1[:, 0:HALF, :], 0.5, n_t[:, 0:HALF, :],
                        ALU.mult, ALU.add,
                    )
                    nc.scalar.activation(u[:, HALF:CH, :], psz23[:], AF.Tanh)
                    nc.vector.scalar_tensor_tensor(
                        t1[:, HALF:CH, :], u[:, HALF:CH, :], 1.0, d[:, HALF:CH, :],
                        ALU.add, ALU.mult,
                    )
                    nc.vector.scalar_tensor_tensor(
                        htb[:, HALF:CH, tsl], t1[:, HALF:CH, :], 0.5,
                        n_t[:, HALF:CH, :], ALU.mult, ALU.add,
                    )
                    hprev = htb[:, :, tsl]

                    if (
                        izin_next is not None
                        and tl >= first_unit_step
                        and (tl - first_unit_step) % 3 == 0
                    ):
                        unit = (tl - first_unit_step) // 3
                        if unit < NUNIT:
                            emit_proj_unit(izin_next, sq_next, unit)

                nc.gpsimd.dma_start(HT[:, :, bsl].rearrange("c p r -> p c r"), htb[:])
                izin_cur = izin_next
                sq_cur = sq_next

    nc.compile()
    return nc


_CACHE: dict = {}


def _get_nc(T, BC):
    key = (T, BC)
    if key not in _CACHE:
        _CACHE[key] = build_nc(T, BC)
    return _CACHE[key]


def make_in_maps(seq, W_iz, b_iz, W_in, b_in, W_hz, b_hz, W_hn, b_hn):
    T, B, _ = seq.shape
    BC = B // NCORES
    f32 = np.float32

    # z gate runs at half scale so z = (tanh(a/2)+1)/2 needs only Tanh.
    wizT = np.ascontiguousarray((W_iz.T * 0.5).astype(NP_BF16))
    winT = np.ascontiguousarray(W_in.T.astype(NP_BF16))
    whzT = np.ascontiguousarray((W_hz.T * 0.5).astype(NP_BF16))
    whnT = np.ascontiguousarray(W_hn.T.astype(NP_BF16))
    biasz = np.ascontiguousarray(((b_iz + b_hz) * 0.5).astype(f32).reshape(CH, 128).T)
    biasn = np.ascontiguousarray((b_in + b_hn).astype(f32).reshape(CH, 128).T)
    identity = np.ascontiguousarray(np.eye(128, dtype=np.float32).astype(NP_BF16))

    in_maps = []
    for c in range(NCORES):
        shard = seq[:, c * BC : (c + 1) * BC, :].reshape(T * BC, DIN)
        seqT = np.ascontiguousarray(shard.T.astype(NP_BF16))
        in_maps.append(
            {
                "seqT": seqT,
                "wizT": wizT,
                "winT": winT,
                "whzT": whzT,
                "whnT": whnT,
                "ident": identity,
                "biasz": biasz,
                "biasn": biasn,
            }
        )
    return in_maps


def run(inputs: dict, trace: bool = False):
    seq = inputs["seq"]
    T, B, _ = seq.shape
    BC = B // NCORES
    nc = _get_nc(T, BC)
    in_maps = make_in_maps(**inputs)
    return nc, bass_utils.run_bass_kernel_spmd(
        nc, in_maps, list(range(NCORES)), trace=trace
    )


def kernel(seq, W_iz, b_iz, W_in, b_in, W_hz, b_hz, W_hn, b_hn):
    T, B, _ = seq.shape
    BC = B // NCORES
    _, res = run(
        dict(
            seq=seq,
            W_iz=W_iz,
            b_iz=b_iz,
            W_in=W_in,
            b_in=b_in,
            W_hz=W_hz,
            b_hz=b_hz,
            W_hn=W_hn,
            b_hn=b_hn,
        )
    )
    out = np.empty((T, B, DH), np.float32)
    for c in range(NCORES):
        HT = res.results[c]["HT"]  # [CH, 128, T*BC] bf16
        Hc = (
            HT.astype(np.float32)
            .reshape(CH, 128, T, BC)
            .transpose(2, 3, 0, 1)
            .reshape(T, BC, DH)
        )
        out[:, c * BC : (c + 1) * BC, :] = Hc
    return out[None]
